# revision 1
# baseline (speedup 1.0000x reference)
"""Density-weighted Manhattan FPS sampler on 8 TRN2 NeuronCores.

Strategy: data-parallel over batch. Each core runs one batch end-to-end
(cores 4-7 duplicate batches 0-3). Two phases per core:

1. Density: pairwise squared-euclidean counts within radius R.
   i-points along 128 partitions (per-partition bias scalars), j-points
   replicated along the free dim; ACT does fused (xj - xi)^2 via
   Square(scale*in + bias); DVE sums components and counts d2 <= R^2 with
   a fused is_le + add-accumulate. Bit-exact vs the XLA reference:
   (dx^2 + dy^2) + dz^2, compare <= f32(0.16000000000000003).

2. FPS loop (2048 sequential steps) on a single 32-partition quadrant,
   entirely on DVE (no cross-engine semaphores in the loop):
   - |x-px|, |y-py|, |2z-2pz| via tensor_scalar (sub, abs_max 0) with the
     winner coords as per-partition scalar APs (cols 0/32/64 of e1t)
   - d = (ax+ay)+az (reference order), min-dist, then ONE
     tensor_tensor_reduce computing key = mdt*pent fused with the per-row
     reduce-max
   - global max + winner-index folds via tensor_reduce(apply_transpose)
     (one op each instead of copy+transpose+reduce)
   - winner coords fetched with a register-indexed dynamic slice from a
     flat x|y|2z copy of the points: ONE broadcast copy [1,3,32] + ONE
     32x32-block transpose makes them per-partition scalars.
   All f32 ops are IEEE-exact so the trajectory matches the reference
   bit-for-bit (required: min argmax margin on this input is ~7e-8).
"""
import numpy as np

import concourse.bacc as bacc
import concourse.bass as bass
import concourse.mybir as mybir
import concourse.tile as tile
from concourse.bass_utils import run_bass_kernel_spmd

F32 = mybir.dt.float32
I32 = mybir.dt.int32
Alu = mybir.AluOpType
Act = mybir.ActivationFunctionType

B = 4
N = 8192
NPOINT = 2048
R2 = float(np.float32(0.16000000000000003))  # f32(0.4*0.4 in f64)
MD_INIT = 1e10

LAST_PERF = None


def build_nc(n=N, npoint=NPOINT, ct=4096, fps_unroll=2, loop_mode="for_i",
             gather_mode="reg", ablate=(), bench_repeats=1):
    """Build the SPMD Bass module. n must be divisible by 256 and ct;
    npoint divisible by fps_unroll."""
    fp, ff = 32, n // 32          # FPS layout [32, ff]
    dp, df = 128, n // 128        # density i-layout [128, df]
    nct = n // ct                 # column tiles per row tile

    nc = bacc.Bacc("TRN2", target_bir_lowering=False, debug=True)

    # --- inputs (host-prepared layouts) ---
    xf_d = nc.dram_tensor("xf", [fp, ff], F32, kind="ExternalInput")
    yf_d = nc.dram_tensor("yf", [fp, ff], F32, kind="ExternalInput")
    z2f_d = nc.dram_tensor("z2f", [fp, ff], F32, kind="ExternalInput")
    iota_d = nc.dram_tensor("iota", [fp, ff], F32, kind="ExternalInput")
    xi_d = nc.dram_tensor("xi", [dp, df], F32, kind="ExternalInput")
    yi_d = nc.dram_tensor("yi", [dp, df], F32, kind="ExternalInput")
    zi_d = nc.dram_tensor("zi", [dp, df], F32, kind="ExternalInput")
    xj_d = nc.dram_tensor("xj", [1, n], F32, kind="ExternalInput")
    yj_d = nc.dram_tensor("yj", [1, n], F32, kind="ExternalInput")
    zj_d = nc.dram_tensor("zj", [1, n], F32, kind="ExternalInput")
    pflat_d = nc.dram_tensor("pflat", [1, 3 * n], F32, kind="ExternalInput")
    seed3_d = nc.dram_tensor("seed3", [1, 3], F32, kind="ExternalInput")

    # --- outputs ---
    idx_out = nc.dram_tensor("idx_out", [npoint], I32, kind="ExternalOutput")
    dens_out = nc.dram_tensor("dens_out", [n], F32, kind="ExternalOutput")

    dens_dram = nc.dram_tensor("dens_dram", [n], F32)

    with tile.TileContext(nc) as tc:
        if True:
            # ---------------- density phase ----------------
            with tc.tile_pool(name="dens", bufs=1) as dpp:
                xi_t = dpp.tile([dp, df], F32)
                yi_t = dpp.tile([dp, df], F32)
                zi_t = dpp.tile([dp, df], F32)
                nc.sync.dma_start(xi_t[:], xi_d[:])
                nc.sync.dma_start(yi_t[:], yi_d[:])
                nc.sync.dma_start(zi_t[:], zi_d[:])
                nxi_t = dpp.tile([dp, df], F32)
                nyi_t = dpp.tile([dp, df], F32)
                nzi_t = dpp.tile([dp, df], F32)
                nc.vector.tensor_scalar(nxi_t[:], xi_t[:], -1.0, None, Alu.mult)
                nc.vector.tensor_scalar(nyi_t[:], yi_t[:], -1.0, None, Alu.mult)
                nc.vector.tensor_scalar(nzi_t[:], zi_t[:], -1.0, None, Alu.mult)

                xj_t = dpp.tile([dp, n], F32)
                yj_t = dpp.tile([dp, n], F32)
                zj_t = dpp.tile([dp, n], F32)
                nc.sync.dma_start(xj_t[:], xj_d[:].broadcast_to((dp, n)))
                nc.sync.dma_start(yj_t[:], yj_d[:].broadcast_to((dp, n)))
                nc.sync.dma_start(zj_t[:], zj_d[:].broadcast_to((dp, n)))

                pcnt = dpp.tile([dp, df * nct], F32)

                with tc.tile_pool(name="dscratch", bufs=2) as sp:
                    for rt in range(df):
                        for c in range(nct):
                            cs = slice(c * ct, (c + 1) * ct)
                            sqx = sp.tile([dp, ct], F32, tag="sqx")
                            sqy = sp.tile([dp, ct], F32, tag="sqy")
                            sqz = sp.tile([dp, ct], F32, tag="sqz")
                            nc.scalar.activation(sqx[:], xj_t[:, cs], Act.Square,
                                                 bias=nxi_t[:, rt:rt + 1], scale=1.0)
                            nc.scalar.activation(sqy[:], yj_t[:, cs], Act.Square,
                                                 bias=nyi_t[:, rt:rt + 1], scale=1.0)
                            nc.scalar.activation(sqz[:], zj_t[:, cs], Act.Square,
                                                 bias=nzi_t[:, rt:rt + 1], scale=1.0)
                            nc.vector.tensor_tensor(sqx[:], sqx[:], sqy[:], Alu.add)
                            nc.vector.tensor_tensor(sqx[:], sqx[:], sqz[:], Alu.add)
                            nc.vector.tensor_scalar(
                                sqy[:], sqx[:], R2, None, Alu.is_le, Alu.add,
                                accum_out=pcnt[:, rt * nct + c: rt * nct + c + 1])

                dens_t = dpp.tile([dp, df], F32)
                if nct > 1:
                    nc.vector.reduce_sum(
                        dens_t[:],
                        pcnt[:].rearrange("p (a b) -> p a b", a=df),
                        axis=mybir.AxisListType.X)
                else:
                    nc.vector.tensor_copy(dens_t[:], pcnt[:])

                # relayout [128, df] (j = rt*128 + p) -> linear dram
                dd2 = dens_dram[:].rearrange("(a b) -> a b", a=df)  # [df, 128]
                nc.sync.dma_start(dd2.transpose([1, 0]), dens_t[:])
                nc.sync.dma_start(dens_out[:], dens_dram[:])

        with tc.tile_pool(name="fps", bufs=1) as pp:
            xf32 = pp.tile([fp, ff], F32)
            yf32 = pp.tile([fp, ff], F32)
            z2f32 = pp.tile([fp, ff], F32)
            iot = pp.tile([fp, ff], F32)
            mdt = pp.tile([fp, ff], F32)
            pent = pp.tile([fp, ff], F32)
            penf = pp.tile([fp, ff], F32)   # raw density in fps layout
            trace = pp.tile([fp, npoint + fps_unroll + 66], I32)

            nc.sync.dma_start(xf32[:], xf_d[:])
            nc.sync.dma_start(yf32[:], yf_d[:])
            nc.sync.dma_start(z2f32[:], z2f_d[:])
            nc.sync.dma_start(iot[:], iota_d[:])

            # load density in fps layout + reciprocal
            nc.sync.dma_start(penf[:], dens_dram[:].rearrange("(a b) -> a b", a=fp))
            nc.vector.reciprocal(pent[:], penf[:])

            # ---------------- FPS init ----------------
            nc.vector.memset(mdt[:], MD_INIT)
            nc.vector.memset(trace[:], 0)

            # ---------------- FPS loop tiles ----------------
            ax = pp.tile([fp, ff], F32)
            ay = pp.tile([fp, ff], F32)
            az = pp.tile([fp, ff], F32)
            s12 = pp.tile([fp, ff], F32)
            dd = pp.tile([fp, ff], F32)
            key = pp.tile([fp, ff], F32)
            junk = pp.tile([fp, ff], F32)
            rowmax = pp.tile([fp, 1], F32)
            mglob = pp.tile([fp, 1], F32)
            e1 = pp.tile([fp, 96], F32)   # row 0 = (px|py|pz2) x32 each
            bt = pp.tile([fp, 96], F32)   # cols 0/32/64 = px/py/pz2 bcast
            e2i = pp.tile([fp, 32], F32)  # col 0 = per-row sum(mask*iota)
            ji = pp.tile([fp, 1], I32)    # partition 0 = winner index

            nc.vector.memset(e1[:], 0.0)
            nc.vector.memset(e2i[:], 0.0)

            seed3 = pp.tile([1, 3], F32)
            nc.sync.dma_start(seed3[:], seed3_d[:])

            # seed with point 0 (host-provided coords)
            nc.vector.tensor_copy(e1[0:1, 0:32],
                                  seed3[0:1, 0:1].broadcast_to((1, 32)))
            nc.vector.tensor_copy(e1[0:1, 32:64],
                                  seed3[0:1, 1:2].broadcast_to((1, 32)))
            nc.vector.tensor_copy(e1[0:1, 64:96],
                                  seed3[0:1, 2:3].broadcast_to((1, 32)))
            nc.vector.transpose(bt[:], e1[:])

            flat3 = pp.tile([1, 3 * n], F32)  # x | y | 2z on partition 0
            nc.sync.dma_start(flat3[:], pflat_d[:])
            jreg = nc.alloc_register(mybir.EngineType.DVE, "jreg")
            jsv = bass.make_scalar_value(
                bass.RegisterHandles([jreg]), min_val=0, max_val=n - 1)
            f3v = flat3[0:1, :].rearrange("a (c n) -> a c n", c=3)
            e1v = e1[0:1, :].rearrange("a (c w) -> a c w", c=3)

            def body(iv):
                # |c - pc| = Abs(-c + pc): three independent ACT ops with
                # per-partition biases from one 32x96 block transpose
                nc.scalar.activation(ax[:], xf32[:], Act.Abs,
                                     bias=bt[:, 0:1], scale=-1.0)
                nc.scalar.activation(ay[:], yf32[:], Act.Abs,
                                     bias=bt[:, 32:33], scale=-1.0)
                if "dveaz" in ablate:
                    # az on DVE, hidden under the ACT window:
                    # azd = z2 - pz2 ; az = max(-azd, azd)
                    nc.vector.tensor_scalar(s12[:], z2f32[:], bt[:, 64:65],
                                            None, Alu.subtract)
                    nc.vector.scalar_tensor_tensor(
                        az[:], s12[:], -1.0, s12[:], op0=Alu.mult, op1=Alu.max)
                else:
                    nc.scalar.activation(az[:], z2f32[:], Act.Abs,
                                         bias=bt[:, 64:65], scale=-1.0)
                # reference-order sum (|dx| + |dy|) + |2dz| and FPS update
                nc.vector.tensor_tensor(s12[:], ax[:], ay[:], Alu.add)
                nc.vector.tensor_tensor(dd[:], s12[:], az[:], Alu.add)
                nc.vector.tensor_tensor(mdt[:], mdt[:], dd[:], Alu.min)
                nc.vector.tensor_tensor(key[:], mdt[:], pent[:], Alu.mult)
                nc.vector.reduce_max(rowmax[:], key[:],
                                     axis=mybir.AxisListType.X)
                # global max on all partitions: transpose-reduce of the
                # broadcast [32,32] row-max block
                nc.vector.tensor_reduce(
                    mglob[:], rowmax[:].broadcast_to((fp, 32)),
                    axis=mybir.AxisListType.X, op=Alu.max,
                    apply_transpose=True)
                # winner index via per-row sum((key >= m) * iota) + col-0 sum
                nc.vector.scalar_tensor_tensor(
                    junk[:], key[:], mglob[:, 0:1], iot[:],
                    op0=Alu.is_ge, op1=Alu.mult,
                    accum_out=e2i[:, 0:1])
                with nc.allow_low_precision(reason="index sums are exact ints"):
                    nc.vector.tensor_reduce(
                        ji[:], e2i[:], axis=mybir.AxisListType.X, op=Alu.add,
                        apply_transpose=True)
                # winner-coordinate gather via register-indexed dynamic slice
                if "noreg" not in ablate:
                    nc.vector.reg_load(jreg, ji[0:1, 0:1])
                    src = f3v[:, :, bass.DynSlice(jsv, 1)].broadcast_to(
                        (1, 3, 32))
                else:
                    src = f3v[:, :, 0:1].broadcast_to((1, 3, 32))
                nc.vector.tensor_copy(e1v, src)
                nc.vector.transpose(bt[:], e1[:])
                # record selected index (output position iv+1)
                if isinstance(iv, int):
                    nc.vector.tensor_copy(
                        trace[0:1, 1 + iv:2 + iv], ji[0:1, 0:1])
                else:
                    nc.vector.tensor_copy(
                        trace[0:1, 1:][:, bass.DynSlice(iv, 1)], ji[0:1, 0:1])

            if loop_mode == "unrolled":
                for t in range(npoint):
                    body(t)
            else:
                for _rep in range(bench_repeats):
                    with tc.For_i(0, npoint, fps_unroll) as iv:
                        for k in range(fps_unroll):
                            body(iv + k)

            # ---------------- outputs ----------------
            idx32 = pp.tile([1, npoint], I32)
            nc.vector.tensor_copy(idx32[:], trace[0:1, 0:npoint])
            nc.sync.dma_start(idx_out[:].rearrange("(a b) -> a b", a=1), idx32[:])

    nc.finalize()
    return nc


def make_in_maps(points, n=N, n_cores=8):
    """Per-core host-side input layouts. Core c handles batch c % B."""
    fp, ff = 32, n // 32
    dp, df = 128, n // 128
    iota = np.arange(n, dtype=np.float32).reshape(fp, ff)
    in_maps = []
    for c in range(n_cores):
        b = c % points.shape[0]
        p = np.ascontiguousarray(points[b])  # [n, 3] f32
        z2 = (p[:, 2] * np.float32(2.0)).astype(np.float32)
        m = {
            "xf": p[:, 0].reshape(fp, ff).copy(),
            "yf": p[:, 1].reshape(fp, ff).copy(),
            "z2f": z2.reshape(fp, ff).copy(),
            "iota": iota,
            "xi": np.ascontiguousarray(p[:, 0].reshape(df, dp).T),
            "yi": np.ascontiguousarray(p[:, 1].reshape(df, dp).T),
            "zi": np.ascontiguousarray(p[:, 2].reshape(df, dp).T),
            "xj": p[:, 0].reshape(1, n).copy(),
            "yj": p[:, 1].reshape(1, n).copy(),
            "zj": p[:, 2].reshape(1, n).copy(),
            "pflat": np.concatenate([p[:, 0], p[:, 1], z2]).reshape(1, 3 * n),
            "seed3": np.array([[p[0, 0], p[0, 1], z2[0]]], np.float32),
        }
        in_maps.append(m)
    return in_maps


_NC_CACHE = {}


def kernel(points, features=None, npoint=None, **_unused):
    """Full-input entry point: points [4, 8192, 3] f32 -> [4, 2048] int32."""
    global LAST_PERF
    points = np.asarray(points, dtype=np.float32)
    assert points.shape == (B, N, 3), points.shape
    npt = int(npoint) if npoint is not None else NPOINT
    assert npt == NPOINT, f"kernel hardcodes npoint={NPOINT}, got {npt}"

    if "nc" not in _NC_CACHE:
        _NC_CACHE["nc"] = build_nc()
    nc = _NC_CACHE["nc"]

    in_maps = make_in_maps(points)
    res = run_bass_kernel_spmd(nc, in_maps, core_ids=list(range(8)))
    LAST_PERF = res
    out = np.stack([res.results[b]["idx_out"] for b in range(B)], axis=0)
    return out.astype(np.int32)



# revision 10
# speedup vs baseline: 1.1682x; 1.1682x over previous
"""Density-weighted Manhattan FPS sampler on 8 TRN2 NeuronCores.

Strategy: data-parallel over batch. Each core runs one batch end-to-end
(cores 4-7 duplicate batches 0-3). Two phases per core:

1. Density: pairwise squared-euclidean counts within radius R.
   i-points along 128 partitions (per-partition bias scalars), j-points
   replicated along the free dim; ACT does fused (xj - xi)^2 via
   Square(scale*in + bias); DVE sums components and counts d2 <= R^2 with
   a fused is_le + add-accumulate. Bit-exact vs the XLA reference:
   (dx^2 + dy^2) + dz^2, compare <= f32(0.16000000000000003).

2. FPS loop (2048 sequential steps) on a single 32-partition quadrant,
   entirely on DVE (no cross-engine semaphores in the loop):
   - |x-px|, |y-py|, |2z-2pz| via tensor_scalar (sub, abs_max 0) with the
     winner coords as per-partition scalar APs (cols 0/32/64 of e1t)
   - d = (ax+ay)+az (reference order), min-dist, then ONE
     tensor_tensor_reduce computing key = mdt*pent fused with the per-row
     reduce-max
   - global max + winner-index folds via tensor_reduce(apply_transpose)
     (one op each instead of copy+transpose+reduce)
   - winner coords fetched with a register-indexed dynamic slice from a
     flat x|y|2z copy of the points: ONE broadcast copy [1,3,32] + ONE
     32x32-block transpose makes them per-partition scalars.
   All f32 ops are IEEE-exact so the trajectory matches the reference
   bit-for-bit (required: min argmax margin on this input is ~7e-8).
"""
import numpy as np

import concourse.bacc as bacc
import concourse.bass as bass
import concourse.mybir as mybir
import concourse.tile as tile
from concourse.bass_utils import run_bass_kernel_spmd

F32 = mybir.dt.float32
I32 = mybir.dt.int32
Alu = mybir.AluOpType
Act = mybir.ActivationFunctionType

B = 4
N = 8192
NPOINT = 2048
R2 = float(np.float32(0.16000000000000003))  # f32(0.4*0.4 in f64)
MD_INIT = 1e10

LAST_PERF = None


def build_nc(n=N, npoint=NPOINT, ct=4096, fps_unroll=2, loop_mode="for_i",
             gather_mode="reg", ablate=(), bench_repeats=1, body_ver=4):
    """Build the SPMD Bass module. n must be divisible by 256 and ct;
    npoint divisible by fps_unroll."""
    fp, ff = 32, n // 32          # FPS layout [32, ff]
    dp, df = 128, n // 128        # density i-layout [128, df]
    nct = n // ct                 # column tiles per row tile

    nc = bacc.Bacc("TRN2", target_bir_lowering=False, debug=True)

    # --- inputs (host-prepared layouts) ---
    xf_d = nc.dram_tensor("xf", [fp, ff], F32, kind="ExternalInput")
    yf_d = nc.dram_tensor("yf", [fp, ff], F32, kind="ExternalInput")
    z2f_d = nc.dram_tensor("z2f", [fp, ff], F32, kind="ExternalInput")
    iota_d = nc.dram_tensor("iota", [fp, ff], F32, kind="ExternalInput")
    xi_d = nc.dram_tensor("xi", [dp, df], F32, kind="ExternalInput")
    yi_d = nc.dram_tensor("yi", [dp, df], F32, kind="ExternalInput")
    zi_d = nc.dram_tensor("zi", [dp, df], F32, kind="ExternalInput")
    xj_d = nc.dram_tensor("xj", [1, n], F32, kind="ExternalInput")
    yj_d = nc.dram_tensor("yj", [1, n], F32, kind="ExternalInput")
    zj_d = nc.dram_tensor("zj", [1, n], F32, kind="ExternalInput")
    pflat_d = nc.dram_tensor("pflat", [1, 3 * n], F32, kind="ExternalInput")
    seed3_d = nc.dram_tensor("seed3", [1, 3], F32, kind="ExternalInput")

    # --- outputs ---
    idx_out = nc.dram_tensor("idx_out", [npoint], I32, kind="ExternalOutput")
    dens_out = nc.dram_tensor("dens_out", [n], F32, kind="ExternalOutput")

    dens_dram = nc.dram_tensor("dens_dram", [n], F32)

    with tile.TileContext(nc) as tc:
        if True:
            # ---------------- density phase ----------------
            with tc.tile_pool(name="dens", bufs=1) as dpp:
                xi_t = dpp.tile([dp, df], F32)
                yi_t = dpp.tile([dp, df], F32)
                zi_t = dpp.tile([dp, df], F32)
                nc.sync.dma_start(xi_t[:], xi_d[:])
                nc.sync.dma_start(yi_t[:], yi_d[:])
                nc.sync.dma_start(zi_t[:], zi_d[:])
                nxi_t = dpp.tile([dp, df], F32)
                nyi_t = dpp.tile([dp, df], F32)
                nzi_t = dpp.tile([dp, df], F32)
                nc.vector.tensor_scalar(nxi_t[:], xi_t[:], -1.0, None, Alu.mult)
                nc.vector.tensor_scalar(nyi_t[:], yi_t[:], -1.0, None, Alu.mult)
                nc.vector.tensor_scalar(nzi_t[:], zi_t[:], -1.0, None, Alu.mult)

                xj_t = dpp.tile([dp, n], F32)
                yj_t = dpp.tile([dp, n], F32)
                zj_t = dpp.tile([dp, n], F32)
                nc.sync.dma_start(xj_t[:], xj_d[:].broadcast_to((dp, n)))
                nc.sync.dma_start(yj_t[:], yj_d[:].broadcast_to((dp, n)))
                nc.sync.dma_start(zj_t[:], zj_d[:].broadcast_to((dp, n)))

                pcnt = dpp.tile([dp, df * nct], F32)

                with tc.tile_pool(name="dscratch", bufs=2) as sp:
                    for rt in range(df):
                        for c in range(nct):
                            cs = slice(c * ct, (c + 1) * ct)
                            sqx = sp.tile([dp, ct], F32, tag="sqx")
                            sqy = sp.tile([dp, ct], F32, tag="sqy")
                            sqz = sp.tile([dp, ct], F32, tag="sqz")
                            nc.scalar.activation(sqx[:], xj_t[:, cs], Act.Square,
                                                 bias=nxi_t[:, rt:rt + 1], scale=1.0)
                            nc.scalar.activation(sqy[:], yj_t[:, cs], Act.Square,
                                                 bias=nyi_t[:, rt:rt + 1], scale=1.0)
                            nc.scalar.activation(sqz[:], zj_t[:, cs], Act.Square,
                                                 bias=nzi_t[:, rt:rt + 1], scale=1.0)
                            nc.vector.tensor_tensor(sqx[:], sqx[:], sqy[:], Alu.add)
                            nc.vector.tensor_tensor(sqx[:], sqx[:], sqz[:], Alu.add)
                            nc.vector.tensor_scalar(
                                sqy[:], sqx[:], R2, None, Alu.is_le, Alu.add,
                                accum_out=pcnt[:, rt * nct + c: rt * nct + c + 1])

                dens_t = dpp.tile([dp, df], F32)
                if nct > 1:
                    nc.vector.reduce_sum(
                        dens_t[:],
                        pcnt[:].rearrange("p (a b) -> p a b", a=df),
                        axis=mybir.AxisListType.X)
                else:
                    nc.vector.tensor_copy(dens_t[:], pcnt[:])

                # relayout [128, df] (j = rt*128 + p) -> linear dram
                dd2 = dens_dram[:].rearrange("(a b) -> a b", a=df)  # [df, 128]
                nc.sync.dma_start(dd2.transpose([1, 0]), dens_t[:])
                nc.sync.dma_start(dens_out[:], dens_dram[:])

        with tc.tile_pool(name="fps", bufs=1) as pp:
            xf32 = pp.tile([fp, ff], F32)
            yf32 = pp.tile([fp, ff], F32)
            z2f32 = pp.tile([fp, ff], F32)
            iot = pp.tile([fp, ff], F32)
            mdt = pp.tile([fp, ff], F32)
            pent = pp.tile([fp, ff], F32)
            penf = pp.tile([fp, ff], F32)   # raw density in fps layout
            trace = pp.tile([fp, npoint + fps_unroll + 66], I32)

            nc.sync.dma_start(xf32[:], xf_d[:])
            nc.sync.dma_start(yf32[:], yf_d[:])
            nc.sync.dma_start(z2f32[:], z2f_d[:])
            nc.sync.dma_start(iot[:], iota_d[:])

            # load density in fps layout + reciprocal
            nc.sync.dma_start(penf[:], dens_dram[:].rearrange("(a b) -> a b", a=fp))
            nc.vector.reciprocal(pent[:], penf[:])

            # ---------------- FPS init ----------------
            nc.vector.memset(mdt[:], MD_INIT)
            nc.vector.memset(trace[:], 0)

            # ---------------- FPS loop tiles ----------------
            ax = pp.tile([fp, ff], F32)
            ay = pp.tile([fp, ff], F32)
            az = pp.tile([fp, ff], F32)
            s12 = pp.tile([fp, ff], F32)
            dd = pp.tile([fp, ff], F32)
            key = pp.tile([fp, ff], F32)
            junk = pp.tile([fp, ff], F32)
            rowmax = pp.tile([fp, 1], F32)
            mglob = pp.tile([fp, 1], F32)
            e1 = pp.tile([fp, 96], F32)   # row 0 = (px|py|pz2) x32 each
            bt = pp.tile([fp, 96], F32)   # cols 0/32/64 = px/py/pz2 bcast
            e2i = pp.tile([fp, 32], F32)  # col 0 = per-row sum(mask*iota)
            ji = pp.tile([fp, 1], I32)    # partition 0 = winner index

            nc.vector.memset(e1[:], 0.0)
            nc.vector.memset(e2i[:], 0.0)
            # body4 reads ji (prev winner; 0 = seed point 0) before its
            # first write — must be initialized
            nc.vector.memset(ji[:], 0)

            jreg = nc.alloc_register(mybir.EngineType.DVE, "jreg")
            jsv = bass.make_scalar_value(
                bass.RegisterHandles([jreg]), min_val=0, max_val=n - 1)
            if body_ver in (1, 2):
                seed3 = pp.tile([1, 3], F32)
                nc.sync.dma_start(seed3[:], seed3_d[:])

                # seed with point 0 (host-provided coords)
                nc.vector.tensor_copy(e1[0:1, 0:32],
                                      seed3[0:1, 0:1].broadcast_to((1, 32)))
                nc.vector.tensor_copy(e1[0:1, 32:64],
                                      seed3[0:1, 1:2].broadcast_to((1, 32)))
                nc.vector.tensor_copy(e1[0:1, 64:96],
                                      seed3[0:1, 2:3].broadcast_to((1, 32)))
                nc.vector.transpose(bt[:], e1[:])

                flat3 = pp.tile([1, 3 * n], F32)  # x | y | 2z on partition 0
                nc.sync.dma_start(flat3[:], pflat_d[:])
                f3v = flat3[0:1, :].rearrange("a (c n) -> a c n", c=3)
                e1v = e1[0:1, :].rearrange("a (c w) -> a c w", c=3)

            def body(iv):
                # |c - pc| = Abs(-c + pc): three independent ACT ops with
                # per-partition biases from one 32x96 block transpose
                nc.scalar.activation(ax[:], xf32[:], Act.Abs,
                                     bias=bt[:, 0:1], scale=-1.0)
                nc.scalar.activation(ay[:], yf32[:], Act.Abs,
                                     bias=bt[:, 32:33], scale=-1.0)
                if "dveaz" in ablate:
                    # az on DVE, hidden under the ACT window:
                    # azd = z2 - pz2 ; az = max(-azd, azd)
                    nc.vector.tensor_scalar(s12[:], z2f32[:], bt[:, 64:65],
                                            None, Alu.subtract)
                    nc.vector.scalar_tensor_tensor(
                        az[:], s12[:], -1.0, s12[:], op0=Alu.mult, op1=Alu.max)
                else:
                    nc.scalar.activation(az[:], z2f32[:], Act.Abs,
                                         bias=bt[:, 64:65], scale=-1.0)
                # reference-order sum (|dx| + |dy|) + |2dz| and FPS update
                nc.vector.tensor_tensor(s12[:], ax[:], ay[:], Alu.add)
                nc.vector.tensor_tensor(dd[:], s12[:], az[:], Alu.add)
                nc.vector.tensor_tensor(mdt[:], mdt[:], dd[:], Alu.min)
                nc.vector.tensor_tensor(key[:], mdt[:], pent[:], Alu.mult)
                nc.vector.reduce_max(rowmax[:], key[:],
                                     axis=mybir.AxisListType.X)
                # global max on all partitions: transpose-reduce of the
                # broadcast [32,32] row-max block
                nc.vector.tensor_reduce(
                    mglob[:], rowmax[:].broadcast_to((fp, 32)),
                    axis=mybir.AxisListType.X, op=Alu.max,
                    apply_transpose=True)
                # winner index via per-row sum((key >= m) * iota) + col-0 sum
                nc.vector.scalar_tensor_tensor(
                    junk[:], key[:], mglob[:, 0:1], iot[:],
                    op0=Alu.is_ge, op1=Alu.mult,
                    accum_out=e2i[:, 0:1])
                with nc.allow_low_precision(reason="index sums are exact ints"):
                    nc.vector.tensor_reduce(
                        ji[:], e2i[:], axis=mybir.AxisListType.X, op=Alu.add,
                        apply_transpose=True)
                # winner-coordinate gather via register-indexed dynamic slice
                if "noreg" not in ablate:
                    nc.vector.reg_load(jreg, ji[0:1, 0:1])
                    src = f3v[:, :, bass.DynSlice(jsv, 1)].broadcast_to(
                        (1, 3, 32))
                else:
                    src = f3v[:, :, 0:1].broadcast_to((1, 3, 32))
                nc.vector.tensor_copy(e1v, src)
                nc.vector.transpose(bt[:], e1[:])
                # record selected index (output position iv+1)
                if isinstance(iv, int):
                    nc.vector.tensor_copy(
                        trace[0:1, 1 + iv:2 + iv], ji[0:1, 0:1])
                else:
                    nc.vector.tensor_copy(
                        trace[0:1, 1:][:, bass.DynSlice(iv, 1)], ji[0:1, 0:1])

            h = ff // 2
            H0, H1 = slice(0, h), slice(h, ff)

            def body2(iv):
                """All-DVE halved chain; az on ACT in parallel. Exact same
                f32 DAG as body(): |dx|,|dy|,|2dz|; (ax+ay)+az; min; *pen;
                global argmax via is_ge-iota-sum."""
                # az on ACT (starts at bt-ready, overlaps DVE ax/ay/s12)
                nc.scalar.activation(az[:], z2f32[:], Act.Abs,
                                     bias=bt[:, 64:65], scale=-1.0)
                # ax, ay on DVE: |c - pc| = (c sub pc) abs_max 0  (2x mode)
                nc.vector.tensor_scalar(ax[:], xf32[:], bt[:, 0:1], 0.0,
                                        Alu.subtract, Alu.abs_max)
                nc.vector.tensor_scalar(ay[:], yf32[:], bt[:, 32:33], 0.0,
                                        Alu.subtract, Alu.abs_max)
                # dependent chain in interleaved halves: each op's input is
                # 2 issues back, so the ~95ns dependency latency is hidden
                for Hs in (H0, H1):
                    nc.vector.tensor_tensor(s12[:, Hs], ax[:, Hs], ay[:, Hs],
                                            Alu.add)
                for Hs in (H0, H1):
                    nc.vector.tensor_tensor(dd[:, Hs], s12[:, Hs], az[:, Hs],
                                            Alu.add)
                for Hs in (H0, H1):
                    nc.vector.tensor_tensor(mdt[:, Hs], mdt[:, Hs], dd[:, Hs],
                                            Alu.min)
                for Hs in (H0, H1):
                    nc.vector.tensor_tensor(key[:, Hs], mdt[:, Hs],
                                            pent[:, Hs], Alu.mult)
                nc.vector.reduce_max(rowmax[:], key[:],
                                     axis=mybir.AxisListType.X)
                nc.vector.tensor_reduce(
                    mglob[:], rowmax[:].broadcast_to((fp, 32)),
                    axis=mybir.AxisListType.X, op=Alu.max,
                    apply_transpose=True)
                nc.vector.scalar_tensor_tensor(
                    junk[:], key[:], mglob[:, 0:1], iot[:],
                    op0=Alu.is_ge, op1=Alu.mult,
                    accum_out=e2i[:, 0:1])
                with nc.allow_low_precision(reason="index sums are exact ints"):
                    nc.vector.tensor_reduce(
                        ji[:], e2i[:], axis=mybir.AxisListType.X, op=Alu.add,
                        apply_transpose=True)
                # trace write first: its exec overlaps the reg_load stall
                if isinstance(iv, int):
                    nc.vector.tensor_copy(
                        trace[0:1, 1 + iv:2 + iv], ji[0:1, 0:1])
                else:
                    nc.vector.tensor_copy(
                        trace[0:1, 1:][:, bass.DynSlice(iv, 1)], ji[0:1, 0:1])
                nc.vector.reg_load(jreg, ji[0:1, 0:1])
                src = f3v[:, :, bass.DynSlice(jsv, 1)].broadcast_to((1, 3, 32))
                nc.vector.tensor_copy(e1v, src)
                nc.vector.transpose(bt[:], e1[:])

            # ---- v4: DynSlice-scalar biases (no gather/transpose), DVE
            # sub+and abs for x/y, ACT az, halved TT chain, TTR-fused
            # key+rowmax. Same f32 DAG as the reference.
            if body_ver == 4:
                cb = pp.tile([fp, 3 * n], F32)  # x | y | 2z bcast to 32 parts
                nc.sync.dma_start(cb[:], pflat_d[:].broadcast_to((fp, 3 * n)))
                cbv = cb[:].rearrange("p (c n) -> p c n", c=3)
            bz = pp.tile([fp, 1], F32)
            rowmax2 = pp.tile([fp, 2], F32)
            rowm = pp.tile([fp, 1], F32)
            ABS_MASK = 0x7FFFFFFF
            U32 = mybir.dt.uint32

            def body4(iv):
                # winner index -> DVE register (ji holds prev winner; 0 at start)
                nc.vector.reg_load(jreg, ji[0:1, 0:1])
                # ACT z-bias: one [32,1] copy, then az on ACT in parallel
                nc.vector.tensor_copy(bz[:], cbv[:, 2:3, bass.DynSlice(jsv, 1)])
                nc.scalar.activation(az[:], z2f32[:], Act.Abs,
                                     bias=bz[:, 0:1], scale=-1.0)
                # |dx|, |dy| on DVE: subtract (DynSlice scalar), then &0x7fffffff
                nc.vector.tensor_scalar(ax[:], xf32[:],
                                        cbv[:, 0:1, bass.DynSlice(jsv, 1)],
                                        None, Alu.subtract)
                nc.vector.tensor_scalar(ay[:], yf32[:],
                                        cbv[:, 1:2, bass.DynSlice(jsv, 1)],
                                        None, Alu.subtract)
                nc.vector.tensor_scalar(ax[:].bitcast(U32), ax[:].bitcast(U32),
                                        ABS_MASK, None, Alu.bitwise_and)
                nc.vector.tensor_scalar(ay[:].bitcast(U32), ay[:].bitcast(U32),
                                        ABS_MASK, None, Alu.bitwise_and)
                # halved dependent chain (deps 2 issues back -> no bubbles)
                for Hs in (H0, H1):
                    nc.vector.tensor_tensor(s12[:, Hs], ax[:, Hs], ay[:, Hs],
                                            Alu.add)
                for Hs in (H0, H1):
                    nc.vector.tensor_tensor(dd[:, Hs], s12[:, Hs], az[:, Hs],
                                            Alu.add)
                for Hs in (H0, H1):
                    nc.vector.tensor_tensor(mdt[:, Hs], mdt[:, Hs], dd[:, Hs],
                                            Alu.min)
                for Hs in (H0, H1):
                    nc.vector.tensor_tensor(key[:, Hs], mdt[:, Hs],
                                            pent[:, Hs], Alu.mult)
                # trace: record prev winner (ji not yet overwritten); slot
                # here so rowmax's dep (key) is 2 issues back -> no bubble
                if isinstance(iv, int):
                    nc.vector.tensor_copy(trace[0:1, iv:iv + 1], ji[0:1, 0:1])
                else:
                    nc.vector.tensor_copy(
                        trace[0:1, bass.DynSlice(iv, 1)], ji[0:1, 0:1])
                nc.vector.reduce_max(rowm[:], key[:],
                                     axis=mybir.AxisListType.X)
                nc.vector.tensor_reduce(
                    mglob[:], rowm[:].broadcast_to((fp, 32)),
                    axis=mybir.AxisListType.X, op=Alu.max,
                    apply_transpose=True)
                nc.vector.scalar_tensor_tensor(
                    junk[:], key[:], mglob[:, 0:1], iot[:],
                    op0=Alu.is_ge, op1=Alu.mult,
                    accum_out=e2i[:, 0:1])
                with nc.allow_low_precision(reason="index sums are exact ints"):
                    nc.vector.tensor_reduce(
                        ji[:], e2i[:], axis=mybir.AxisListType.X, op=Alu.add,
                        apply_transpose=True)

            fps_body = {1: body, 2: body2, 4: body4}[body_ver]

            if loop_mode == "unrolled":
                for t in range(npoint):
                    fps_body(t)
            else:
                for _rep in range(bench_repeats):
                    with tc.For_i(0, npoint, fps_unroll) as iv:
                        for k in range(fps_unroll):
                            fps_body(iv + k)

            # ---------------- outputs ----------------
            idx32 = pp.tile([1, npoint], I32)
            nc.vector.tensor_copy(idx32[:], trace[0:1, 0:npoint])
            nc.sync.dma_start(idx_out[:].rearrange("(a b) -> a b", a=1), idx32[:])

    nc.finalize()
    return nc


def make_in_maps(points, n=N, n_cores=8):
    """Per-core host-side input layouts. Core c handles batch c % B."""
    fp, ff = 32, n // 32
    dp, df = 128, n // 128
    iota = np.arange(n, dtype=np.float32).reshape(fp, ff)
    in_maps = []
    for c in range(n_cores):
        b = c % points.shape[0]
        p = np.ascontiguousarray(points[b])  # [n, 3] f32
        z2 = (p[:, 2] * np.float32(2.0)).astype(np.float32)
        m = {
            "xf": p[:, 0].reshape(fp, ff).copy(),
            "yf": p[:, 1].reshape(fp, ff).copy(),
            "z2f": z2.reshape(fp, ff).copy(),
            "iota": iota,
            "xi": np.ascontiguousarray(p[:, 0].reshape(df, dp).T),
            "yi": np.ascontiguousarray(p[:, 1].reshape(df, dp).T),
            "zi": np.ascontiguousarray(p[:, 2].reshape(df, dp).T),
            "xj": p[:, 0].reshape(1, n).copy(),
            "yj": p[:, 1].reshape(1, n).copy(),
            "zj": p[:, 2].reshape(1, n).copy(),
            "pflat": np.concatenate([p[:, 0], p[:, 1], z2]).reshape(1, 3 * n),
            "seed3": np.array([[p[0, 0], p[0, 1], z2[0]]], np.float32),
        }
        in_maps.append(m)
    return in_maps


_NC_CACHE = {}


def kernel(points, features=None, npoint=None, **_unused):
    """Full-input entry point: points [4, 8192, 3] f32 -> [4, 2048] int32."""
    global LAST_PERF
    points = np.asarray(points, dtype=np.float32)
    assert points.shape == (B, N, 3), points.shape
    npt = int(npoint) if npoint is not None else NPOINT
    assert npt == NPOINT, f"kernel hardcodes npoint={NPOINT}, got {npt}"

    if "nc" not in _NC_CACHE:
        _NC_CACHE["nc"] = build_nc()
    nc = _NC_CACHE["nc"]

    in_maps = make_in_maps(points)
    res = run_bass_kernel_spmd(nc, in_maps, core_ids=list(range(8)))
    LAST_PERF = res
    out = np.stack([res.results[b]["idx_out"] for b in range(B)], axis=0)
    return out.astype(np.int32)

